# revision 27
# baseline (speedup 1.0000x reference)
"""MatchAttention fused forward (l1_norm) on 8 Trainium2 NeuronCores.

Strategy:
  - Data-parallel over (batch=2) x (4 row-bands of N=4096): 8 shards, one per core.
  - Host precomputes, per (b, head): an edge-replicated padded K|V image
    (32 k-ch | 32 v-ch interleaved per pixel, fp16), stored TWICE as
    128-element "pixel pair" rows — copy 0 pair-aligned even, copy 1 shifted
    by one pixel — so any 8-pixel patch row is 4 consecutive 128-elem rows
    in one of the copies (dma_gather needs elem_step % 256 bytes == 0).
    Clamping the window center to [-r, H-1+r] + edge-replicated padding
    reproduces the reference's per-pixel clip exactly.
  - Device, per 128-query tile x head: one dma_gather pulls the 7 patch rows
    (512 fp16 each) per query into that query's SBUF partition; DVE computes
    d = kg - q (fp16), sim = -reduce_abs_add(d) (fp32), ACT exponentiates,
    DVE normalizes (softmax over the 49 window positions), multiplies by the
    gathered V and reduces over the window -> out.
"""

import numpy as np

# Problem constants (hardcoded per contest contract).
B, H, W, C, NH = 2, 64, 64, 256, 8
N = H * W               # 4096
CH = C // NH            # 32 channels per head
R = 3                   # r_h == r_w
KW = 2 * R + 1          # 7
K = KW * KW             # 49
PAD = 6                 # top/left pad; equals 2*R
HP = H + 2 * PAD        # 76
WP = W + PAD + 10       # 80 (right pad 10 so 8-px gather rows never overrun)
WPAIR = WP // 2         # 40 pixel-pairs per image row
ROWE = 2 * 64           # 128 elems per pixel-pair row
HROWS = 2 * HP * WPAIR  # 6080 pixel-pair rows per head (2 parity copies)
GROW = 8 * 64           # 512 elems gathered per index (8 px x 64 ch)
NCORES = 8
BANDS_PER_B = 4
BAND = N // BANDS_PER_B  # 1024
PTILE = 128
TILES = BAND // PTILE    # 8
NIDX = PTILE * KW        # 896 gather indices per (tile, head)
IDXW = NIDX // 16        # 56 wrapped-index columns

_cache = {}


def _build_nc():
    """Build + compile the per-core Bass program (same program on all cores)."""
    if "nc" in _cache:
        return _cache["nc"]
    import bass_rust
    import concourse.tile as tile
    from concourse import bacc, mybir

    f16, f32, i16 = mybir.dt.float16, mybir.dt.float32, mybir.dt.int16
    X = mybir.AxisListType.X
    ADD = mybir.AluOpType.add
    SUB = mybir.AluOpType.subtract
    MUL = mybir.AluOpType.mult

    nc = bacc.Bacc("TRN2", target_bir_lowering=False, debug=False,
                   num_devices=NCORES)

    kv_h = nc.dram_tensor("kv", [NH * HROWS * ROWE], f16, kind="ExternalInput")
    q_d = nc.dram_tensor("q", [BAND, C], f16, kind="ExternalInput").ap()
    idx_d = nc.dram_tensor("idx", [BAND, NH * IDXW], i16, kind="ExternalInput").ap()
    out_d = nc.dram_tensor("out", [BAND, C], f32, kind="ExternalOutput").ap()
    attn_d = nc.dram_tensor("attn", [BAND, NH * K], f32, kind="ExternalOutput").ap()

    # Per-head gather source: overlapping rows of 512 elems at 128-elem steps.
    # Row count 6077 keeps the last row's 512-elem payload inside the head
    # slice (max used start row is 6074).
    kv_src = [bass_rust.AP(kv_h, g * HROWS * ROWE, [[ROWE, HROWS - 3], [1, GROW]])
              for g in range(NH)]

    with tile.TileContext(nc) as tc, \
         tc.tile_pool(name="kvp", bufs=2) as kvp, \
         tc.tile_pool(name="work", bufs=2) as wp, \
         tc.tile_pool(name="soft", bufs=3) as sp, \
         tc.tile_pool(name="io", bufs=3) as iop:
        HSZ = KW * GROW                      # 3584 elems per head per query
        for t in range(TILES):
            rows = slice(t * PTILE, (t + 1) * PTILE)
            qtile = iop.tile([PTILE, C], f16, tag="q")
            nc.sync.dma_start(out=qtile[:], in_=q_d[rows, :])
            itile = iop.tile([PTILE, NH * IDXW], i16, tag="idx")
            nc.sync.dma_start(out=itile[:], in_=idx_d[rows, :])

            # Gather the 7x8px KV patch for every (query, head) into one big
            # per-tile SBUF tile so compute can batch across heads.
            kvbig = kvp.tile([PTILE, NH * HSZ], f16, tag="kv")
            for g in range(NH):
                nc.gpsimd.dma_gather(
                    out_ap=kvbig[:, g * HSZ:(g + 1) * HSZ]
                        .rearrange("p (a b) -> p a b", a=KW),
                    in_ap=kv_src[g],
                    idxs_ap=itile[:, g * IDXW:(g + 1) * IDXW],
                    num_idxs=NIDX,
                    num_idxs_reg=NIDX,
                    elem_size=GROW,
                    elem_step=ROWE,
                )
            kv5 = kvbig[:].rearrange("p (g a b c) -> p g a b c", g=NH, a=KW, b=8)
            kview = kv5[:, :, :, 0:KW, 0:CH]         # [128, 8, 7, 7, 32]
            vview = kv5[:, :, :, 0:KW, CH:2 * CH]

            # sim[q, g, k] = sum_c |kg - q|  (one sub + one abs-reduce, all heads)
            qb = (qtile[:].rearrange("p (g c) -> p g c", g=NH)
                  .unsqueeze(2).unsqueeze(2)
                  .to_broadcast([PTILE, NH, KW, KW, CH]))
            d = wp.tile([PTILE, NH * K * CH], f16, tag="d")
            d5 = d[:].rearrange("p (g a b c) -> p g a b c", g=NH, a=KW, b=KW)
            dabs = d[:].rearrange("p (g k c) -> p g k c", g=NH, c=CH)
            sim = sp.tile([PTILE, NH * K], f32, tag="sim")
            sim3 = sim[:].rearrange("p (g k) -> p g k", g=NH)
            e = sp.tile([PTILE, NH * K], f32, tag="e")
            e3 = e[:].rearrange("p (g k) -> p g k", g=NH)
            s = sp.tile([PTILE, NH], f32, tag="s")
            rcp = sp.tile([PTILE, NH], f32, tag="r")
            attn = sp.tile([PTILE, NH * K], f32, tag="attn")
            attn3 = attn[:].rearrange("p (g k) -> p g k", g=NH)
            a16x = wp.tile([PTILE, NH * K * CH], f16, tag="d")
            a16x4 = a16x[:].rearrange("p (g k c) -> p g k c", g=NH, c=CH)
            a5 = a16x[:].rearrange("p (g a b c) -> p g a b c", g=NH, a=KW, b=KW)
            GH2 = NH // 8
            for h in range(8):
                gs = slice(h * GH2, (h + 1) * GH2)
                for g in range(gs.start, gs.stop):
                    # per-head: the ISA allows at most 3 free dims and the
                    # strided K|V view burns them all.
                    nc.vector.tensor_tensor(out=d5[:, g], in0=kview[:, g],
                                            in1=qb[:, g], op=SUB)
                # |d| on ACT (in place), two fp16 folds over c, then reduce
                dflat = d[:].rearrange("p (g x) -> p g x", g=NH)
                nc.scalar.activation(out=dflat[:, gs], in_=dflat[:, gs],
                                     func=mybir.ActivationFunctionType.Abs)
                nc.vector.tensor_tensor(
                    out=dabs[:, gs, :, 0:16], in0=dabs[:, gs, :, 0:16],
                    in1=dabs[:, gs, :, 16:32], op=ADD)
                nc.vector.tensor_tensor(
                    out=dabs[:, gs, :, 0:8], in0=dabs[:, gs, :, 0:8],
                    in1=dabs[:, gs, :, 8:16], op=ADD)
                nc.vector.tensor_reduce(
                    out=sim3[:, gs, :], in_=dabs[:, gs, :, 0:8],
                    axis=X, op=ADD)
                # softmax over k within each head. sim in [0, ~90] so
                # exp(-sim) is safe in fp32 without max-subtraction.
                # per-head exp on ACT; accum_out yields the softmax
                # denominator without a DVE reduce
                for g in range(gs.start, gs.stop):
                    nc.scalar.activation(out=e3[:, g, :], in_=sim3[:, g, :],
                                         func=mybir.ActivationFunctionType.Exp,
                                         scale=-1.0, accum_out=s[:, g:g + 1])
                nc.vector.reciprocal(out=rcp[:, gs], in_=s[:, gs])
                for g in range(gs.start, gs.stop):
                    nc.scalar.activation(out=attn3[:, g, :], in_=e3[:, g, :],
                                         func=mybir.ActivationFunctionType.Copy,
                                         scale=rcp[:, g:g + 1])
                nc.scalar.copy(
                    out=a16x4[:, gs, :, :],
                    in_=attn3[:, gs, :].unsqueeze(3)
                        .to_broadcast([PTILE, GH2, K, CH]))
                # out = sum_k attn * vg : fp16 mul in place into a16x
                for g in range(gs.start, gs.stop):
                    nc.vector.tensor_tensor(out=a5[:, g], in0=vview[:, g],
                                            in1=a5[:, g], op=MUL)
            nc.sync.dma_start(out=attn_d[rows, :], in_=attn[:])
            p4 = a16x[:].rearrange("p (g k c) -> p g k c", g=NH, c=CH)
            for lo, hi, n in ((0, 25, 24), (0, 13, 12), (0, 7, 6), (0, 4, 3)):
                nc.vector.tensor_tensor(
                    out=p4[:, :, lo:lo + n, :],
                    in0=p4[:, :, lo:lo + n, :],
                    in1=p4[:, :, hi:hi + n, :],
                    op=ADD)
            nc.vector.tensor_tensor(out=p4[:, :, 0:2, :],
                                    in0=p4[:, :, 0:2, :],
                                    in1=p4[:, :, 2:4, :], op=ADD)
            ot = iop.tile([PTILE, C], f32, tag="o")
            nc.vector.tensor_tensor(
                out=ot[:].rearrange("p (g c) -> p g c", g=NH),
                in0=p4[:, :, 0, :], in1=p4[:, :, 1, :], op=ADD)
            nc.sync.dma_start(out=out_d[rows, :], in_=ot[:])

    nc.compile()
    _cache["nc"] = nc
    return nc


def make_core_inputs(max_offset, q, k, v):
    """Host-side preprocessing: parity-duplicated padded fp16 KV images, fp16
    q, wrapped int16 gather indices; returns per-core input dicts
    (core c -> batch c//4, band c%4)."""
    off = np.round(np.asarray(max_offset, np.float32)).astype(np.int64)  # [B,N,h,2]
    ys = (np.arange(N) // W)[None, :, None]
    xs = (np.arange(N) % W)[None, :, None]
    cy = np.clip(ys + off[..., 0], -R, H - 1 + R)        # [B,N,h]
    cx = np.clip(xs + off[..., 1], -R, W - 1 + R)
    srow = (cy + R).astype(np.int64)                     # [0, 69] padded row start
    scol = (cx + R).astype(np.int64)                     # [0, 69] padded col start
    dy = np.arange(KW)[None, None, None, :]
    s = (srow[..., None] + dy) * WP + scol[..., None]    # [B,N,h,7]
    par = s & 1
    row16 = par * (HP * WPAIR) + (s - par) // 2          # [B,N,h,7] in [0, 6074]
    assert row16.max() < HROWS - 3

    # wrapped int16 index layout per (band-tile, head):
    #   flat[j*128 + p] = row16[query p, head g, dy j]
    #   wrapped[pp, ss] = flat[ss*16 + pp], replicated to 128 partitions.
    idx_host = np.empty((B, N, NH * IDXW), np.int16)
    r5 = row16.reshape(B, N // PTILE, PTILE, NH, KW)
    for t in range(N // PTILE):
        blk = r5[:, t]                                   # [B, 128, NH, 7]
        flat = blk.transpose(0, 2, 3, 1).reshape(B, NH, NIDX)   # j*128+p order
        wrapped = flat.reshape(B, NH, IDXW, 16).transpose(0, 1, 3, 2)  # [B,NH,16,56]
        rep = np.tile(wrapped, (1, 1, 8, 1))             # [B, NH, 128, 56]
        idx_host[:, t * PTILE:(t + 1) * PTILE, :] = (
            rep.transpose(0, 2, 1, 3).reshape(B, PTILE, NH * IDXW))

    # padded K|V images -> parity-duplicated pair rows
    kh = np.asarray(k, np.float32).reshape(B, H, W, NH, CH).transpose(0, 3, 1, 2, 4)
    vh = np.asarray(v, np.float32).reshape(B, H, W, NH, CH).transpose(0, 3, 1, 2, 4)
    kvi = np.concatenate([kh, vh], axis=-1)              # [B, NH, H, W, 64]
    kvi = np.pad(kvi, ((0, 0), (0, 0), (PAD, PAD), (PAD, WP - W - PAD), (0, 0)),
                 mode="edge").astype(np.float16)         # [B, NH, HP, WP, 64]
    copy0 = kvi.reshape(B, NH, HP, WPAIR, ROWE)
    ext = np.concatenate([kvi, kvi[:, :, :, -1:, :]], axis=3)  # [B,NH,HP,WP+1,64]
    copy1 = ext[:, :, :, 1:WP + 1, :].reshape(B, NH, HP, WPAIR, ROWE)
    kv2 = np.stack([copy0, copy1], axis=2)               # [B, NH, 2, HP, WPAIR, ROWE]
    kv2 = np.ascontiguousarray(kv2).reshape(B, NH * HROWS * ROWE)

    q16 = np.asarray(q, np.float16)                      # [B, N, C]

    in_maps = []
    for c in range(NCORES):
        b, band = divmod(c, BANDS_PER_B)
        sl = slice(band * BAND, (band + 1) * BAND)
        in_maps.append({
            "kv": kv2[b],
            "q": np.ascontiguousarray(q16[b, sl]),
            "idx": np.ascontiguousarray(idx_host[b, sl]),
        })
    return in_maps


def kernel(max_offset, q, k, v, H=H, W=W, r_h=R, r_w=R, _trace=False):
    assert int(H) == 64 and int(W) == 64 and int(r_h) == 3 and int(r_w) == 3
    from concourse.bass_utils import run_bass_kernel_spmd

    nc = _build_nc()
    in_maps = make_core_inputs(max_offset, q, k, v)
    res = run_bass_kernel_spmd(nc, in_maps, core_ids=list(range(NCORES)),
                               trace=_trace)
    _cache["last_results"] = res

    output = np.empty((B, N, C), np.float32)
    attn_out = np.empty((B, N, NH, K), np.float32)
    for c in range(NCORES):
        b, band = divmod(c, BANDS_PER_B)
        sl = slice(band * BAND, (band + 1) * BAND)
        output[b, sl] = res.results[c]["out"]
        attn_out[b, sl] = res.results[c]["attn"].reshape(BAND, NH, K)
    return output, attn_out


# revision 28
# speedup vs baseline: 1.0022x; 1.0022x over previous
"""MatchAttention fused forward (l1_norm) on 8 Trainium2 NeuronCores.

Strategy:
  - Data-parallel over (batch=2) x (4 row-bands of N=4096): 8 shards, one per core.
  - Host precomputes, per (b, head): an edge-replicated padded K|V image
    (32 k-ch | 32 v-ch interleaved per pixel, fp16), stored TWICE as
    128-element "pixel pair" rows — copy 0 pair-aligned even, copy 1 shifted
    by one pixel — so any 8-pixel patch row is 4 consecutive 128-elem rows
    in one of the copies (dma_gather needs elem_step % 256 bytes == 0).
    Clamping the window center to [-r, H-1+r] + edge-replicated padding
    reproduces the reference's per-pixel clip exactly.
  - Device, per 128-query tile x head: one dma_gather pulls the 7 patch rows
    (512 fp16 each) per query into that query's SBUF partition; DVE computes
    d = kg - q (fp16), sim = -reduce_abs_add(d) (fp32), ACT exponentiates,
    DVE normalizes (softmax over the 49 window positions), multiplies by the
    gathered V and reduces over the window -> out.
"""

import numpy as np

# Problem constants (hardcoded per contest contract).
B, H, W, C, NH = 2, 64, 64, 256, 8
N = H * W               # 4096
CH = C // NH            # 32 channels per head
R = 3                   # r_h == r_w
KW = 2 * R + 1          # 7
K = KW * KW             # 49
PAD = 6                 # top/left pad; equals 2*R
HP = H + 2 * PAD        # 76
WP = W + PAD + 10       # 80 (right pad 10 so 8-px gather rows never overrun)
WPAIR = WP // 2         # 40 pixel-pairs per image row
ROWE = 2 * 64           # 128 elems per pixel-pair row
HROWS = 2 * HP * WPAIR  # 6080 pixel-pair rows per head (2 parity copies)
GROW = 8 * 64           # 512 elems gathered per index (8 px x 64 ch)
NCORES = 8
BANDS_PER_B = 4
BAND = N // BANDS_PER_B  # 1024
PTILE = 128
TILES = BAND // PTILE    # 8
NIDX = PTILE * KW        # 896 gather indices per (tile, head)
IDXW = NIDX // 16        # 56 wrapped-index columns

_cache = {}


def _build_nc():
    """Build + compile the per-core Bass program (same program on all cores)."""
    if "nc" in _cache:
        return _cache["nc"]
    import bass_rust
    import concourse.tile as tile
    from concourse import bacc, mybir

    f16, f32, i16 = mybir.dt.float16, mybir.dt.float32, mybir.dt.int16
    X = mybir.AxisListType.X
    ADD = mybir.AluOpType.add
    SUB = mybir.AluOpType.subtract
    MUL = mybir.AluOpType.mult

    nc = bacc.Bacc("TRN2", target_bir_lowering=False, debug=False,
                   num_devices=NCORES)

    kv_h = nc.dram_tensor("kv", [NH * HROWS * ROWE], f16, kind="ExternalInput")
    q_d = nc.dram_tensor("q", [BAND, C], f16, kind="ExternalInput").ap()
    idx_d = nc.dram_tensor("idx", [BAND, NH * IDXW], i16, kind="ExternalInput").ap()
    out_d = nc.dram_tensor("out", [BAND, C], f32, kind="ExternalOutput").ap()
    attn_d = nc.dram_tensor("attn", [BAND, NH * K], f32, kind="ExternalOutput").ap()

    # Per-head gather source: overlapping rows of 512 elems at 128-elem steps.
    # Row count 6077 keeps the last row's 512-elem payload inside the head
    # slice (max used start row is 6074).
    kv_src = [bass_rust.AP(kv_h, g * HROWS * ROWE, [[ROWE, HROWS - 3], [1, GROW]])
              for g in range(NH)]

    with tile.TileContext(nc) as tc, \
         tc.tile_pool(name="kvp", bufs=2) as kvp, \
         tc.tile_pool(name="work", bufs=2) as wp, \
         tc.tile_pool(name="soft", bufs=3) as sp, \
         tc.tile_pool(name="io", bufs=3) as iop:
        HSZ = KW * GROW                      # 3584 elems per head per query
        for t in range(TILES):
            rows = slice(t * PTILE, (t + 1) * PTILE)
            qtile = iop.tile([PTILE, C], f16, tag="q")
            nc.sync.dma_start(out=qtile[:], in_=q_d[rows, :])
            itile = iop.tile([PTILE, NH * IDXW], i16, tag="idx")
            nc.sync.dma_start(out=itile[:], in_=idx_d[rows, :])

            # Gather the 7x8px KV patch for every (query, head) into one big
            # per-tile SBUF tile so compute can batch across heads.
            kvbig = kvp.tile([PTILE, NH * HSZ], f16, tag="kv")
            for g in range(NH):
                nc.gpsimd.dma_gather(
                    out_ap=kvbig[:, g * HSZ:(g + 1) * HSZ]
                        .rearrange("p (a b) -> p a b", a=KW),
                    in_ap=kv_src[g],
                    idxs_ap=itile[:, g * IDXW:(g + 1) * IDXW],
                    num_idxs=NIDX,
                    num_idxs_reg=NIDX,
                    elem_size=GROW,
                    elem_step=ROWE,
                )
            kv5 = kvbig[:].rearrange("p (g a b c) -> p g a b c", g=NH, a=KW, b=8)
            kview = kv5[:, :, :, 0:KW, 0:CH]         # [128, 8, 7, 7, 32]
            vview = kv5[:, :, :, 0:KW, CH:2 * CH]

            # sim[q, g, k] = sum_c |kg - q|  (one sub + one abs-reduce, all heads)
            qb = (qtile[:].rearrange("p (g c) -> p g c", g=NH)
                  .unsqueeze(2).unsqueeze(2)
                  .to_broadcast([PTILE, NH, KW, KW, CH]))
            d = wp.tile([PTILE, NH * K * CH], f16, tag="d")
            d5 = d[:].rearrange("p (g a b c) -> p g a b c", g=NH, a=KW, b=KW)
            dabs = d[:].rearrange("p (g k c) -> p g k c", g=NH, c=CH)
            sim = sp.tile([PTILE, NH * K], f32, tag="sim")
            sim3 = sim[:].rearrange("p (g k) -> p g k", g=NH)
            e = sp.tile([PTILE, NH * K], f32, tag="e")
            e3 = e[:].rearrange("p (g k) -> p g k", g=NH)
            s = sp.tile([PTILE, NH], f32, tag="s")
            rcp = sp.tile([PTILE, NH], f32, tag="r")
            attn = sp.tile([PTILE, NH * K], f32, tag="attn")
            attn3 = attn[:].rearrange("p (g k) -> p g k", g=NH)
            a16x = wp.tile([PTILE, NH * K * CH], f16, tag="d")
            a16x4 = a16x[:].rearrange("p (g k c) -> p g k c", g=NH, c=CH)
            a5 = a16x[:].rearrange("p (g a b c) -> p g a b c", g=NH, a=KW, b=KW)
            GH2 = NH // 8
            # all subs issued first: deep DVE ready-queue hides ACT waits
            for g in range(NH):
                nc.vector.tensor_tensor(out=d5[:, g], in0=kview[:, g],
                                        in1=qb[:, g], op=SUB)
            for h in range(8):
                gs = slice(h * GH2, (h + 1) * GH2)
                # |d| on ACT (in place), two fp16 folds over c, then reduce
                dflat = d[:].rearrange("p (g x) -> p g x", g=NH)
                nc.scalar.activation(out=dflat[:, gs], in_=dflat[:, gs],
                                     func=mybir.ActivationFunctionType.Abs)
                nc.vector.tensor_tensor(
                    out=dabs[:, gs, :, 0:16], in0=dabs[:, gs, :, 0:16],
                    in1=dabs[:, gs, :, 16:32], op=ADD)
                nc.vector.tensor_tensor(
                    out=dabs[:, gs, :, 0:8], in0=dabs[:, gs, :, 0:8],
                    in1=dabs[:, gs, :, 8:16], op=ADD)
                nc.vector.tensor_reduce(
                    out=sim3[:, gs, :], in_=dabs[:, gs, :, 0:8],
                    axis=X, op=ADD)
                # softmax over k within each head. sim in [0, ~90] so
                # exp(-sim) is safe in fp32 without max-subtraction.
                # per-head exp on ACT; accum_out yields the softmax
                # denominator without a DVE reduce
                for g in range(gs.start, gs.stop):
                    nc.scalar.activation(out=e3[:, g, :], in_=sim3[:, g, :],
                                         func=mybir.ActivationFunctionType.Exp,
                                         scale=-1.0, accum_out=s[:, g:g + 1])
                nc.vector.reciprocal(out=rcp[:, gs], in_=s[:, gs])
                for g in range(gs.start, gs.stop):
                    nc.scalar.activation(out=attn3[:, g, :], in_=e3[:, g, :],
                                         func=mybir.ActivationFunctionType.Copy,
                                         scale=rcp[:, g:g + 1])
                nc.scalar.copy(
                    out=a16x4[:, gs, :, :],
                    in_=attn3[:, gs, :].unsqueeze(3)
                        .to_broadcast([PTILE, GH2, K, CH]))
                # out = sum_k attn * vg : fp16 mul in place into a16x
                for g in range(gs.start, gs.stop):
                    nc.vector.tensor_tensor(out=a5[:, g], in0=vview[:, g],
                                            in1=a5[:, g], op=MUL)
            nc.sync.dma_start(out=attn_d[rows, :], in_=attn[:])
            p4 = a16x[:].rearrange("p (g k c) -> p g k c", g=NH, c=CH)
            for lo, hi, n in ((0, 25, 24), (0, 13, 12), (0, 7, 6), (0, 4, 3)):
                nc.vector.tensor_tensor(
                    out=p4[:, :, lo:lo + n, :],
                    in0=p4[:, :, lo:lo + n, :],
                    in1=p4[:, :, hi:hi + n, :],
                    op=ADD)
            nc.vector.tensor_tensor(out=p4[:, :, 0:2, :],
                                    in0=p4[:, :, 0:2, :],
                                    in1=p4[:, :, 2:4, :], op=ADD)
            ot = iop.tile([PTILE, C], f32, tag="o")
            nc.vector.tensor_tensor(
                out=ot[:].rearrange("p (g c) -> p g c", g=NH),
                in0=p4[:, :, 0, :], in1=p4[:, :, 1, :], op=ADD)
            nc.sync.dma_start(out=out_d[rows, :], in_=ot[:])

    nc.compile()
    _cache["nc"] = nc
    return nc


def make_core_inputs(max_offset, q, k, v):
    """Host-side preprocessing: parity-duplicated padded fp16 KV images, fp16
    q, wrapped int16 gather indices; returns per-core input dicts
    (core c -> batch c//4, band c%4)."""
    off = np.round(np.asarray(max_offset, np.float32)).astype(np.int64)  # [B,N,h,2]
    ys = (np.arange(N) // W)[None, :, None]
    xs = (np.arange(N) % W)[None, :, None]
    cy = np.clip(ys + off[..., 0], -R, H - 1 + R)        # [B,N,h]
    cx = np.clip(xs + off[..., 1], -R, W - 1 + R)
    srow = (cy + R).astype(np.int64)                     # [0, 69] padded row start
    scol = (cx + R).astype(np.int64)                     # [0, 69] padded col start
    dy = np.arange(KW)[None, None, None, :]
    s = (srow[..., None] + dy) * WP + scol[..., None]    # [B,N,h,7]
    par = s & 1
    row16 = par * (HP * WPAIR) + (s - par) // 2          # [B,N,h,7] in [0, 6074]
    assert row16.max() < HROWS - 3

    # wrapped int16 index layout per (band-tile, head):
    #   flat[j*128 + p] = row16[query p, head g, dy j]
    #   wrapped[pp, ss] = flat[ss*16 + pp], replicated to 128 partitions.
    idx_host = np.empty((B, N, NH * IDXW), np.int16)
    r5 = row16.reshape(B, N // PTILE, PTILE, NH, KW)
    for t in range(N // PTILE):
        blk = r5[:, t]                                   # [B, 128, NH, 7]
        flat = blk.transpose(0, 2, 3, 1).reshape(B, NH, NIDX)   # j*128+p order
        wrapped = flat.reshape(B, NH, IDXW, 16).transpose(0, 1, 3, 2)  # [B,NH,16,56]
        rep = np.tile(wrapped, (1, 1, 8, 1))             # [B, NH, 128, 56]
        idx_host[:, t * PTILE:(t + 1) * PTILE, :] = (
            rep.transpose(0, 2, 1, 3).reshape(B, PTILE, NH * IDXW))

    # padded K|V images -> parity-duplicated pair rows
    kh = np.asarray(k, np.float32).reshape(B, H, W, NH, CH).transpose(0, 3, 1, 2, 4)
    vh = np.asarray(v, np.float32).reshape(B, H, W, NH, CH).transpose(0, 3, 1, 2, 4)
    kvi = np.concatenate([kh, vh], axis=-1)              # [B, NH, H, W, 64]
    kvi = np.pad(kvi, ((0, 0), (0, 0), (PAD, PAD), (PAD, WP - W - PAD), (0, 0)),
                 mode="edge").astype(np.float16)         # [B, NH, HP, WP, 64]
    copy0 = kvi.reshape(B, NH, HP, WPAIR, ROWE)
    ext = np.concatenate([kvi, kvi[:, :, :, -1:, :]], axis=3)  # [B,NH,HP,WP+1,64]
    copy1 = ext[:, :, :, 1:WP + 1, :].reshape(B, NH, HP, WPAIR, ROWE)
    kv2 = np.stack([copy0, copy1], axis=2)               # [B, NH, 2, HP, WPAIR, ROWE]
    kv2 = np.ascontiguousarray(kv2).reshape(B, NH * HROWS * ROWE)

    q16 = np.asarray(q, np.float16)                      # [B, N, C]

    in_maps = []
    for c in range(NCORES):
        b, band = divmod(c, BANDS_PER_B)
        sl = slice(band * BAND, (band + 1) * BAND)
        in_maps.append({
            "kv": kv2[b],
            "q": np.ascontiguousarray(q16[b, sl]),
            "idx": np.ascontiguousarray(idx_host[b, sl]),
        })
    return in_maps


def kernel(max_offset, q, k, v, H=H, W=W, r_h=R, r_w=R, _trace=False):
    assert int(H) == 64 and int(W) == 64 and int(r_h) == 3 and int(r_w) == 3
    from concourse.bass_utils import run_bass_kernel_spmd

    nc = _build_nc()
    in_maps = make_core_inputs(max_offset, q, k, v)
    res = run_bass_kernel_spmd(nc, in_maps, core_ids=list(range(NCORES)),
                               trace=_trace)
    _cache["last_results"] = res

    output = np.empty((B, N, C), np.float32)
    attn_out = np.empty((B, N, NH, K), np.float32)
    for c in range(NCORES):
        b, band = divmod(c, BANDS_PER_B)
        sl = slice(band * BAND, (band + 1) * BAND)
        output[b, sl] = res.results[c]["out"]
        attn_out[b, sl] = res.results[c]["attn"].reshape(BAND, NH, K)
    return output, attn_out
